# revision 1
# baseline (speedup 1.0000x reference)
"""A3C loss kernel for Trainium2 (8 NeuronCores, data-parallel over batch).

The reference is a reverse scan over T=128 timesteps per trajectory:
    R_t   = sum_{s>=t} g^(s-t) r_s + g^(T-t) R0
    gae_t = sum_{s>=t} g^(s-t) delta_s,  delta_s = r_s + g*v_{s+1} - v_s (v_T = R0)
    critic = 0.5 * sum_t (R_t - v_t)^2
    actor  = -sum_t lp_t * gae_t - beta * sum_{t,a} ent
Both suffix scans are matmuls with a [T,T] discount matrix, so the whole
loss becomes DMA + one A-reduction + two small matmuls per 128-row block.

Layout: each core owns BC=8192 rows; block k (0..63), partition p holds local
row p*64+k, which makes the final [BC,2] output a single contiguous DMA from a
[128, 128] staging tile.
"""

import numpy as np
from contextlib import ExitStack

import concourse.bacc as bacc
import concourse.bass as bass
import concourse.tile as tile
from concourse import mybir
from concourse.bass_utils import run_bass_kernel_spmd

GAMMA = 0.99
BETA = 0.01
B, T, A = 65536, 128, 8
N_CORES = 8
BC = B // N_CORES

F32 = mybir.dt.float32
ALU = mybir.AluOpType
ACTF = mybir.ActivationFunctionType


def _discount_matrix() -> np.ndarray:
    # L[s, t] = gamma^(s-t) for s >= t else 0
    s = np.arange(T, dtype=np.float64)[:, None]
    t = np.arange(T, dtype=np.float64)[None, :]
    m = np.where(s >= t, GAMMA ** np.maximum(s - t, 0.0), 0.0)
    return m.astype(np.float32)


def build_nc(bc: int = BC):
    kb = bc // 128
    assert bc % 128 == 0

    nc = bacc.Bacc("TRN2", target_bir_lowering=False, debug=False)

    v_d = nc.dram_tensor("values", [bc, T], F32, kind="ExternalInput")
    lv_d = nc.dram_tensor("last_value", [bc], F32, kind="ExternalInput")
    r_d = nc.dram_tensor("rewards", [bc, T], F32, kind="ExternalInput")
    lp_d = nc.dram_tensor("log_probs", [bc, T, A], F32, kind="ExternalInput")
    en_d = nc.dram_tensor("entropies", [bc, T, A], F32, kind="ExternalInput")
    tm_d = nc.dram_tensor("terminal_mask", [bc], mybir.dt.uint8, kind="ExternalInput")
    out_d = nc.dram_tensor("out", [bc, 2], F32, kind="ExternalOutput")

    lgam_d = nc.inline_tensor(_discount_matrix(), "lgam")
    iden_d = nc.inline_tensor(np.eye(128, dtype=np.float32), "iden")

    v_view = v_d.rearrange("(p k) t -> k p t", k=kb)
    r_view = r_d.rearrange("(p k) t -> k p t", k=kb)
    lp_view = lp_d.rearrange("(p k) t a -> k p t a", k=kb)
    en_view = en_d.rearrange("(p k) t a -> k p (t a)", k=kb)
    lv_view = lv_d.rearrange("(p k) -> p k", k=kb)
    tm_view = tm_d.rearrange("(p k) -> p k", k=kb)
    out_view = out_d.rearrange("(p k) j -> p (k j)", k=kb)

    with tile.TileContext(nc) as tc, ExitStack() as ctx:
        singles = ctx.enter_context(tc.tile_pool(name="singles", bufs=1))
        work = ctx.enter_context(tc.tile_pool(name="work", bufs=6))
        big = ctx.enter_context(tc.tile_pool(name="big", bufs=5))
        scr = ctx.enter_context(tc.tile_pool(name="scr", bufs=2))
        psum = ctx.enter_context(tc.tile_pool(name="psum", bufs=3, space="PSUM"))

        # singles go through SWDGE (gpsimd) so the SP HWDGE FIFO starts on
        # the block loads immediately
        lgam_s = singles.tile([128, 128], F32)
        nc.gpsimd.dma_start(out=lgam_s, in_=lgam_d[:])
        iden_s = singles.tile([128, 128], F32)
        nc.gpsimd.dma_start(out=iden_s, in_=iden_d[:])
        lv_s = singles.tile([128, kb], F32)
        nc.gpsimd.dma_start(out=lv_s, in_=lv_view)
        tm_s = singles.tile([128, kb], mybir.dt.uint8)
        nc.gpsimd.dma_start(out=tm_s, in_=tm_view)

        # gr0 = gamma * last_value * (1 - mask)
        tmf = singles.tile([128, kb], F32)
        nc.gpsimd.tensor_copy(out=tmf, in_=tm_s)
        lvm = singles.tile([128, kb], F32)
        nc.gpsimd.tensor_mul(lvm, lv_s, tmf)
        gr0 = singles.tile([128, kb], F32)
        nc.gpsimd.tensor_sub(gr0, lv_s, lvm)
        nc.gpsimd.tensor_scalar_mul(gr0, gr0, GAMMA)

        stage = singles.tile([128, 2 * kb], F32)

        for k in range(kb):
            lpb = big.tile([128, T, A], F32)
            nc.sync.dma_start(out=lpb, in_=lp_view[k])
            enb = big.tile([128, T * A], F32)
            nc.sync.dma_start(out=enb, in_=en_view[k])
            v = work.tile([128, T], F32)
            nc.sync.dma_start(out=v, in_=v_view[k])
            r = work.tile([128, T], F32)
            nc.sync.dma_start(out=r, in_=r_view[k])

            # lp[b, t] = sum_a log_probs  (frees lpb early so its big-pool
            # slot recycles for DMA prefetch)
            lp = work.tile([128, T], F32)
            nc.vector.reduce_sum(out=lp, in_=lpb, axis=mybir.AxisListType.X)

            # nbe[b] = -beta * sum_{t,a} entropies
            entscr = scr.tile([128, T * A], F32)
            nbe = work.tile([128, 1], F32)
            nc.scalar.activation(
                out=entscr, in_=enb, func=ACTF.Copy, bias=0.0, scale=-BETA,
                accum_out=nbe,
            )

            # r' = r with gamma*R0 folded into the last timestep
            nc.gpsimd.tensor_tensor(
                out=r[:, T - 1 : T], in0=r[:, T - 1 : T], in1=gr0[:, k : k + 1],
                op=ALU.add,
            )

            # time-major copy of r' for the scan matmul
            rT_ps = psum.tile([128, T], F32)
            nc.tensor.transpose(rT_ps, r, iden_s)
            rT = work.tile([128, T], F32)
            nc.vector.tensor_copy(out=rT, in_=rT_ps)

            # R[b, t] = sum_s r'T[s, b] * Lgam[s, t]  (batch-major result)
            R_ps = psum.tile([128, T], F32)
            nc.tensor.matmul(R_ps, lhsT=rT, rhs=lgam_s, start=True, stop=True)

            # gae_t telescopes to exactly adv_t = R_t - v_t (lambda=1 GAE)
            adv = work.tile([128, T], F32)
            nc.vector.tensor_sub(adv, R_ps, v)

            # critic = 0.5 * sum_t adv^2
            sq = work.tile([128, T], F32)
            nc.scalar.activation(
                out=sq, in_=adv, func=ACTF.Square, bias=0.0,
                scale=float(np.sqrt(0.5)),
                accum_out=stage[:, 2 * k + 1 : 2 * k + 2],
            )

            # actor = -sum_t lp*adv - beta*sum ent
            # (tensor_tensor_reduce crashes this runtime; STT + accum_out works)
            prod = work.tile([128, T], F32)
            acc = work.tile([128, 1], F32)
            nc.vector.scalar_tensor_tensor(
                out=prod, in0=adv, scalar=-1.0, in1=lp,
                op0=ALU.mult, op1=ALU.mult, accum_out=acc,
            )
            nc.gpsimd.tensor_tensor(
                out=stage[:, 2 * k : 2 * k + 1], in0=acc, in1=nbe, op=ALU.add,
            )

            # stream the staged outputs out in chunks so the final store
            # overlaps the tail of the main loop
        # single store at the end, on the second HWDGE ring so it stays out of
        # the SP FIFO that carries the input loads
        nc.scalar.dma_start(out=out_view, in_=stage)

    nc.compile()
    return nc


_NC = None


def _get_nc():
    global _NC
    if _NC is None:
        _NC = build_nc(BC)
    return _NC


def _make_in_maps(inputs: dict) -> list[dict]:
    v = np.ascontiguousarray(np.asarray(inputs["values"], dtype=np.float32))
    lv = np.ascontiguousarray(np.asarray(inputs["last_value"], dtype=np.float32))
    r = np.ascontiguousarray(np.asarray(inputs["rewards"], dtype=np.float32))
    lp = np.ascontiguousarray(np.asarray(inputs["log_probs"], dtype=np.float32))
    en = np.ascontiguousarray(np.asarray(inputs["entropies"], dtype=np.float32))
    tm = np.ascontiguousarray(np.asarray(inputs["terminal_mask"]).astype(np.uint8))
    maps = []
    for c in range(N_CORES):
        sl = slice(c * BC, (c + 1) * BC)
        maps.append(
            {
                "values": v[sl],
                "last_value": lv[sl],
                "rewards": r[sl],
                "log_probs": lp[sl],
                "entropies": en[sl],
                "terminal_mask": tm[sl],
            }
        )
    return maps


def _run(inputs: dict, trace: bool = False):
    nc = _get_nc()
    res = run_bass_kernel_spmd(
        nc,
        _make_in_maps(inputs),
        core_ids=list(range(N_CORES)),
        trace=trace,
    )
    out = np.concatenate([res.results[c]["out"] for c in range(N_CORES)], axis=0)
    return out, res


def kernel(**inputs) -> np.ndarray:
    out, _ = _run(inputs, trace=False)
    return out

